# revision 1
# baseline (speedup 1.0000x reference)
"""Trainium2 Bass kernel for nn_DenoisingSharpening (v2: TensorE offload).

Contract: kernel(**inputs) takes the FULL unsharded inputs
(images [8,64,64,64,3] f32, params [8,64,7] f32, k [] f32) and returns
the FULL output [8,64,64,64,3] f32.

Strategy
--------
Data-parallel over N = B*P = 512 images; 64 images per NeuronCore, one
half-image (32 rows) per SBUF partition -> 128 partitions, 4 row-chunks.

v2 moves every *linear combination* onto the (otherwise idle) PE array via
PSUM accumulation with identity / per-partition-diagonal f16 weights:
  * bilateral numerator: 8 shifted windows of the 4 ck*diff product fields
    accumulate as +I / -I matmuls (replaces DVE mirror-subtract + adds).
  * bilateral denominator: 8 shifted windows of the 4 ck fields as +I
    matmuls into one PSUM bank.
  * gaussian detail: inner = (1-2bE)*Hmid + bE*(H_N+H_S) + dN_S - dN via
    diag(1-2bE)/diag(bE)/+-I matmuls (replaces the DVE 5-op chain).
Everything elementwise runs f16 where alignment allows (2x/4x DVE modes),
with the remaining TT work split between DVE and GPSIMD by a cost balance.
"""

import numpy as np

N_CORES = 8
B, PP, H, W, C = 8, 64, 64, 64, 3
NIMG = B * PP  # 512
HALVES = 2 * NIMG  # 1024 half-images, 128 per core
PR, PC = 34, 66  # padded half-image rows/cols
ROWS_PER_HALF = 32
CHUNKS = 4
CR = ROWS_PER_HALF // CHUNKS  # interior rows per chunk (8)
SLAB_R = CR + 2  # slab rows incl. halo (10)
RP = CR // 2  # row-pairs per chunk (4)

NOISE_THRESH = 0.002
SKIP_THRESH = 1e-4
MEAN_N = float(C * H * W)  # 12288 elements per image mean

# params columns
(P_S, P_LOGE, P_LOGC, P_WSC, P_BE, P_KT, P_KTB, P_IGT, P_OFFGT, P_CLIP,
 P_SQL, P_NSQL, P_TN, P_1M2BE, P_PAD1, P_PAD2) = range(16)
NPARAM = 16

_CACHE = {}


# --------------------------------------------------------------------------
# host-side preprocessing
# --------------------------------------------------------------------------

def _host_prep(images, params, k):
    x = np.ascontiguousarray(images, dtype=np.float32).reshape(NIMG, H, W, C)
    xp = np.pad(x, ((0, 0), (1, 1), (1, 1), (0, 0)), mode="reflect")
    # halves: rows 0..33 and 32..65 of the padded [66,66,3]
    halves = np.stack([xp[:, 0:PR], xp[:, ROWS_PER_HALF:ROWS_PER_HALF + PR]], axis=1)
    halves = np.ascontiguousarray(halves, dtype=np.float32).reshape(HALVES, PR, PC, C)

    p = np.asarray(params, dtype=np.float32).reshape(NIMG, 7)
    sigma_s = np.clip(p[:, 0], 0.2, 5.0)
    sigma_r = np.clip(p[:, 1], 0.01, 1.0)
    sigma_f = np.clip(p[:, 2], 0.2, 3.0)
    lam = np.clip(p[:, 3], 0.1, 2.0)
    tau = np.clip(p[:, 4], 0.5, 5.0)
    gain = np.clip(p[:, 5], 0.2, 2.0)
    offset = np.clip(p[:, 6], 0.01, 1.0)

    def gauss1d(sig):
        g = np.exp(-0.5 * (np.array([-1.0, 0.0, 1.0], np.float32)[None, :] / sig[:, None]) ** 2)
        return g / g.sum(axis=1, keepdims=True)

    gs = gauss1d(sigma_s)  # [N,3]: [aE, aC, aE]
    gf = gauss1d(sigma_f)
    aE, aC = gs[:, 0], gs[:, 1]
    bE = gf[:, 0]

    kpos = max(abs(float(np.asarray(k))), 1.0)
    gt = gain / tau

    pars = np.zeros((NIMG, NPARAM), np.float32)
    pars[:, P_S] = np.sqrt(0.5) / sigma_r
    pars[:, P_LOGE] = np.log(aE * aC)
    pars[:, P_LOGC] = np.log(aE * aE)
    pars[:, P_WSC] = aC * aC
    pars[:, P_BE] = bE
    pars[:, P_KT] = 0.5 * kpos
    pars[:, P_KTB] = -0.5 * kpos * NOISE_THRESH
    pars[:, P_IGT] = 1.0 / gt
    pars[:, P_OFFGT] = offset / gt
    pars[:, P_CLIP] = 10.0 / tau
    pars[:, P_SQL] = np.sqrt(lam * bE / 2.0)
    pars[:, P_NSQL] = -np.sqrt(lam * bE / 2.0)
    pars[:, P_TN] = MEAN_N * SKIP_THRESH / tau
    pars[:, P_1M2BE] = 1.0 - 2.0 * bE
    # duplicate per half-image
    pars2 = np.repeat(pars, 2, axis=0)  # [1024, NPARAM]

    in_maps = []
    per_core = HALVES // N_CORES
    for c in range(N_CORES):
        sl = slice(c * per_core, (c + 1) * per_core)
        in_maps.append({
            "xpad": np.ascontiguousarray(halves[sl]),
            "pp": np.ascontiguousarray(pars2[sl]),
        })
    return in_maps


def _host_post(results, images=None, params=None):
    outs = [r["out"] for r in results]  # each [128, 32, 64, 3]
    full = np.concatenate(outs, axis=0)  # [1024, 32, 64, 3]
    full = full.reshape(NIMG, 2, ROWS_PER_HALF, W, C).reshape(NIMG, H, W, C)
    return full.reshape(B, PP, H, W, C)


# --------------------------------------------------------------------------
# device program
# --------------------------------------------------------------------------

def build_program(cfg=None):
    import concourse.tile as tile
    from concourse import bacc, mybir
    from contextlib import ExitStack

    cfg = cfg or {}
    F32 = mybir.dt.float32
    F16 = mybir.dt.float16
    ALU = mybir.AluOpType
    AF = mybir.ActivationFunctionType
    repeat = int(cfg.get("repeat", 1))
    # engine routing knobs ('dve' or 'pool')
    hf_eng = cfg.get("hf_eng", "dve")
    ne0_eng = cfg.get("ne0_eng", "dve")
    o2b_eng = cfg.get("o2b_eng", "dve")
    d1_eng = cfg.get("d1_eng", "dve")
    s3_split = bool(cfg.get("s3_split", True))
    ck3 = bool(cfg.get("ck3", True))
    tts_eng = cfg.get("tts_eng", "pool")
    o3_eng = cfg.get("o3_eng", "dve")
    gauss_first = bool(cfg.get("gauss_first", True))
    sacc_pad = bool(cfg.get("sacc_pad", False))
    tt_split = bool(cfg.get("tt_split", False))
    inn_split = bool(cfg.get("inn_split", False))
    # taps routed to gpsimd (tuples of tap names)
    pool_diff = cfg.get("pool_diff", ("NE", "NW"))
    pool_prod = cfg.get("pool_prod", ())
    pool_d2 = cfg.get("pool_d2", ())

    nc = bacc.Bacc("TRN2", target_bir_lowering=False, debug=False)
    xdram = nc.dram_tensor("xpad", [128, PR, PC, C], F32, kind="ExternalInput").ap()
    pdram = nc.dram_tensor("pp", [128, NPARAM], F32, kind="ExternalInput").ap()
    odram = nc.dram_tensor("out", [128, ROWS_PER_HALF, W, C], F32, kind="ExternalOutput").ap()

    # taps: name, dr, dc, row slice (slab coords), col slice, bias col
    # W rows are extended to the full slab so the gaussian can reuse its diffs.
    TAPS = [
        ("W", 0, -1, (0, 10), (1, 66), P_LOGE),
        ("NW", -1, -1, (1, 10), (1, 66), P_LOGC),
        ("N", -1, 0, (1, 10), (1, 65), P_LOGE),
        ("NE", -1, 1, (1, 10), (0, 65), P_LOGC),
    ]
    # rows actually needed for the bilateral part of each tap (slab coords)
    BIL_ROWS = {"W": (1, 9), "NW": (1, 10), "N": (1, 10), "NE": (1, 10)}

    with tile.TileContext(nc) as tc:
        with ExitStack() as ctx:
            pool = ctx.enter_context(tc.tile_pool(name="main", bufs=1))
            psp = ctx.enter_context(tc.tile_pool(name="ps", bufs=1, space="PSUM"))

            pp = pool.tile([128, NPARAM], F32, tag="pp", bufs=1)
            nc.sync.dma_start(pp[:], pdram[:])

            def par(col):
                return pp[:, col:col + 1]

            # ---- PE weights: identity / -identity / diag(bE) / diag(1-2bE)
            ident = pool.tile([128, 128], F16, tag="wid", bufs=1, name="ident")
            nc.gpsimd.memset(ident[:], 0.0)
            nc.gpsimd.affine_select(
                out=ident[:], in_=ident[:],
                compare_op=ALU.not_equal, fill=1.0, base=0,
                pattern=[[-1, 128]], channel_multiplier=1)
            nident = pool.tile([128, 128], F16, tag="wnid", bufs=1, name="nident")
            nc.gpsimd.memset(nident[:], 0.0)
            nc.gpsimd.affine_select(
                out=nident[:], in_=nident[:],
                compare_op=ALU.not_equal, fill=-1.0, base=0,
                pattern=[[-1, 128]], channel_multiplier=1)
            wbe = pool.tile([128, 128], F16, tag="wbe", bufs=1, name="wbe")
            nc.vector.tensor_scalar(wbe[:], ident[:], par(P_BE), None, ALU.mult)
            w1m2 = pool.tile([128, 128], F16, tag="w1m2", bufs=1, name="w1m2")
            nc.vector.tensor_scalar(w1m2[:], ident[:], par(P_1M2BE), None, ALU.mult)

            for rep in range(repeat):
              accs = pool.tile([128, 2 * CHUNKS], F32, tag="accs", bufs=1,
                               name=f"accs{rep}")
              tts_s = []

              for ch in range(CHUNKS):
                rbase = ch * CR
                slab = pool.tile([128, SLAB_R, PC, C], F32, tag="slab", bufs=2,
                                 name=f"slab{ch}")
                nc.sync.dma_start(slab[:], xdram[:, rbase:rbase + SLAB_R, :, :])
                x_int = slab[:, 1:1 + CR, 1:65, :]

                # ---------------- per-tap fields (f16) ----------------
                d16s = {}
                cks = {}
                wins = {}  # name -> (ir0, ic0, mr0, mc0)
                for (name, dr, dc, (rs, re), (cs, ce), bcol) in TAPS:
                    nr, ncol = re - rs, ce - cs
                    brs, bre = BIL_ROWS[name]
                    bnr = bre - brs
                    boff = brs - rs
                    diff = pool.tile([128, SLAB_R, PC, C], F16, tag="df" + name,
                                     bufs=(int(cfg.get("dwn_bufs", 2))
                                           if name in ("W", "N") else 1),
                                     name=f"df{name}{ch}")
                    # W/N diffs feed the gaussian late in the chunk -> bufs=2;
                    # NW/NE die right after their product -> bufs=1
                    dv = diff[:, 0:nr, 0:ncol, :]
                    deng = nc.gpsimd if name in pool_diff else nc.vector
                    deng.tensor_tensor(
                        dv, slab[:, rs + dr:re + dr, cs + dc:ce + dc, :],
                        slab[:, rs:re, cs:ce, :], ALU.subtract)
                    d16s[name] = diff
                    # squares (ACT, scale = sqrt(.5)/sigma_r folded)
                    bdv = diff[:, boff:boff + bnr, 0:ncol, :]
                    sq = pool.tile([128, SLAB_R - 1, PC, C], F16, tag="sq",
                                   bufs=int(cfg.get("sq_bufs", 3)),
                                   name=f"sq{name}{ch}")
                    sv = sq[:, 0:bnr, 0:ncol, :]
                    nc.scalar.activation(sv, bdv, AF.Square, scale=par(P_S))
                    # channel sum -> d2
                    d2eng = nc.gpsimd if name in pool_d2 else nc.vector
                    d2a = pool.tile([128, SLAB_R - 1, PC], F16, tag="d2a",
                                    bufs=int(cfg.get("d2_bufs", 2)),
                                    name=f"d2a{name}{ch}")
                    av = d2a[:, 0:bnr, 0:ncol]
                    d2eng.tensor_tensor(
                        av, sq[:, 0:bnr, 0:ncol, 0], sq[:, 0:bnr, 0:ncol, 1],
                        ALU.add)
                    d2 = pool.tile([128, SLAB_R - 1, PC], F16, tag="d2",
                                   bufs=int(cfg.get("d2_bufs", 2)),
                                   name=f"d2{name}{ch}")
                    bv = d2[:, 0:bnr, 0:ncol]
                    d2eng.tensor_tensor(bv, av, sq[:, 0:bnr, 0:ncol, 2], ALU.add)
                    # color kernel (f16). With ck3, ACT broadcast-writes all 3
                    # channels so the product TT is an aligned f16 2x op.
                    # all cks stay live until the denominator matmuls
                    if ck3:
                        ck = pool.tile([128, SLAB_R - 1, PC, C], F16, tag="ck",
                                       bufs=int(cfg.get("ck_bufs", 4)),
                                       name=f"ck{name}{ch}")
                        ckv = ck[:, 0:bnr, 0:ncol, :]
                        nc.scalar.activation(
                            ckv,
                            bv.unsqueeze(3).broadcast_to([128, bnr, ncol, C]),
                            AF.Exp, bias=par(bcol), scale=-1.0)
                        ck_mul = ckv
                        cks[name] = ck[:, :, :, 0]
                    else:
                        ck = pool.tile([128, SLAB_R - 1, PC], F16, tag="ck",
                                       bufs=8, name=f"ck{name}{ch}")
                        ckv = ck[:, 0:bnr, 0:ncol]
                        nc.scalar.activation(ckv, bv, AF.Exp, bias=par(bcol),
                                             scale=-1.0)
                        ck_mul = ckv.unsqueeze(3).broadcast_to([128, bnr, ncol, C])
                        cks[name] = ck
                    # product field (f16)
                    prod = pool.tile([128, SLAB_R - 1, PC, C], F16, tag="pr" + name,
                                     bufs=int(cfg.get("prod_bufs", 2)),
                                     name=f"pr{name}{ch}")
                    pv = prod[:, 0:bnr, 0:ncol, :]
                    peng = nc.gpsimd if name in pool_prod else nc.vector
                    peng.tensor_tensor(
                        pv, diff[:, boff:boff + bnr, 0:ncol, :], ck_mul, ALU.mult)
                    d16s[name + "_prod"] = prod
                    ir0, ic0 = 1 - brs, 1 - cs
                    wins[name] = (ir0, ic0, ir0 - dr, ic0 - dc)

                # psum_big rows padded to 256 f32 so each row-pair is one bank;
                # reused sequentially: one of {numerator, gaussian} then the
                # other (order set by gauss_first). With sacc_pad, the
                # denominator lives in the padding cols (192:256) of the same
                # banks — matmul start=True clears has_written BITS bank-wide
                # but not VALUES, so a region written by a completed group
                # survives later groups in the same bank. That frees the
                # separate sacc bank and allows big bufs=2 (8 banks exactly),
                # overlapping chunk n+1's matmuls with chunk n's consumers.
                big = psp.tile([128, CR, 256], F32, tag="big",
                               bufs=2 if sacc_pad else 1, name=f"big{ch}")
                if sacc_pad:
                    sacc = big[:, :, 192:256]
                else:
                    sacc_t = psp.tile([128, CR, W], F32, tag="sacc", bufs=2,
                                      name=f"sacc{ch}")
                    sacc = sacc_t[:]
                    i_mm = 0
                    for (name, _, _, _, _, _) in TAPS:
                        ck = cks[name]
                        ir0, ic0, mr0, mc0 = wins[name]
                        for r0, c0 in ((ir0, ic0), (mr0, mc0)):
                            rhs = ck[:, r0:r0 + CR, c0:c0 + W]
                            nc.tensor.matmul(
                                sacc, ident[:], rhs,
                                start=(i_mm == 0), stop=(i_mm == 7))
                            i_mm += 1
                tT = pool.tile([128, CR, W, C], F16, tag="tT",
                               bufs=int(cfg.get("tt_bufs", 2)),
                               name=f"tT{ch}")
                inn16 = pool.tile([128, CR, W, C], F16, tag="inn",
                                  bufs=int(cfg.get("inn_bufs", 2)),
                                  name=f"inn{ch}")

                def emit_numerator():
                    # ------------- PE: numerator accumulation -------------
                    for k in range(RP):
                        if sacc_pad:
                            # denominator group for this bank first: its
                            # region (cols 192:256) is untouched by the
                            # numerator group that follows, so the values
                            # survive that group's bank-wide bit clear.
                            s_ap = big[:, 2 * k:2 * k + 2, 192:256]
                            i_mm = 0
                            for (name, _, _, _, _, _) in TAPS:
                                ck = cks[name]
                                ir0, ic0, mr0, mc0 = wins[name]
                                for r0, c0 in ((ir0, ic0), (mr0, mc0)):
                                    rhs = ck[:, r0 + 2 * k:r0 + 2 * k + 2,
                                             c0:c0 + W]
                                    nc.tensor.matmul(
                                        s_ap, ident[:], rhs,
                                        start=(i_mm == 0), stop=(i_mm == 7))
                                    i_mm += 1
                        out_ap = big[:, 2 * k:2 * k + 2, 0:192]
                        n_mm = 2 * len(TAPS)
                        i_mm = 0
                        for sign, widx in ((0, ident), (1, nident)):
                            for (name, _, _, _, _, _) in TAPS:
                                prod = d16s[name + "_prod"]
                                ir0, ic0, mr0, mc0 = wins[name]
                                r0 = (ir0 if sign == 0 else mr0) + 2 * k
                                c0 = ic0 if sign == 0 else mc0
                                rhs = prod[:, r0:r0 + 2, c0:c0 + W, :]
                                nc.tensor.matmul(
                                    out_ap, widx[:], rhs,
                                    start=(i_mm == 0), stop=(i_mm == n_mm - 1))
                                i_mm += 1
                    # + center weight, then reciprocal
                    saccw = pool.tile([128, CR, W], F32, tag="saccw",
                                      bufs=int(cfg.get("saccw_bufs", 1)),
                                      name=f"saccw{ch}")
                    nc.scalar.activation(saccw[:], sacc, AF.Identity,
                                         bias=par(P_WSC))
                    rS = pool.tile([128, CR, W], F32, tag="rS",
                                   bufs=int(cfg.get("rs_bufs", 1)),
                                   name=f"rS{ch}")
                    nc.vector.reciprocal_approx_fast(
                        rS[:].rearrange("p a b -> p (a b)"),
                        saccw[:].rearrange("p a b -> p (a b)"))
                    if tt_split:
                        # per-bank reads start as soon as that bank's
                        # accumulation group stops (shorter PSUM tail)
                        for k in range(RP):
                            r = slice(2 * k, 2 * k + 2)
                            nc.vector.tensor_tensor(
                                tT[:, r, :, :],
                                big[:, r, 0:192].rearrange(
                                    "p r (w c) -> p r w c", c=3),
                                rS[:, r, :].unsqueeze(3).broadcast_to(
                                    [128, 2, W, C]),
                                ALU.mult)
                    else:
                        nc.vector.tensor_tensor(
                            tT[:],
                            big[:, :, 0:192].rearrange("p r (w c) -> p r w c",
                                                       c=3),
                            rS[:].unsqueeze(3).broadcast_to([128, CR, W, C]),
                            ALU.mult)

                def emit_gaussian():
                    # ------- gaussian detail: H field + PE combine -------
                    dW, dN = d16s["W"], d16s["N"]
                    Hf = pool.tile([128, SLAB_R, W, C], F16, tag="gau", bufs=2,
                                   name=f"H{ch}")
                    heng = nc.gpsimd if hf_eng == "pool" else nc.vector
                    heng.tensor_tensor(
                        Hf[:], dW[:, 0:SLAB_R, 1:W + 1, :],
                        dW[:, 0:SLAB_R, 0:W, :], ALU.subtract)
                    # inner = (1-2bE)*Hmid + bE*H_N + bE*H_S + dN_A - dN_B
                    for k in range(RP):
                        out_ap = big[:, 2 * k:2 * k + 2, 0:192]
                        terms = [
                            (wbe, Hf[:, 2 * k:2 + 2 * k, :, :]),
                            (wbe, Hf[:, 2 + 2 * k:4 + 2 * k, :, :]),
                            (w1m2, Hf[:, 1 + 2 * k:3 + 2 * k, :, :]),
                            (ident, dN[:, 1 + 2 * k:3 + 2 * k, 0:W, :]),
                            (nident, dN[:, 2 * k:2 + 2 * k, 0:W, :]),
                        ]
                        for i, (widx, rhs) in enumerate(terms):
                            nc.tensor.matmul(
                                out_ap, widx[:], rhs,
                                start=(i == 0), stop=(i == len(terms) - 1))
                    inner = big[:, :, 0:192].rearrange("p r (w c) -> p r w c",
                                                       c=3)
                    # copy inner out of PSUM right away so the single 'big'
                    # psum buffer frees early for the other accumulation
                    if inn_split:
                        for k in range(RP):
                            r = slice(2 * k, 2 * k + 2)
                            nc.scalar.activation(
                                inn16[:, r, :, :],
                                big[:, r, 0:192].rearrange(
                                    "p r (w c) -> p r w c", c=3),
                                AF.Copy)
                    else:
                        nc.scalar.activation(inn16[:], inner, AF.Copy)

                if gauss_first:
                    emit_gaussian()
                    emit_numerator()
                else:
                    emit_numerator()
                    emit_gaussian()

                # ---------------- noise / masks (f16 chain) ----------------
                # tag nzA rotation: adet, th, nm, s3 (th must survive to s3)
                # tag nzB rotation: ne0, neq, sqn, ee, t2 (chain distance 1)
                adet = pool.tile([128, CR, W, C], F16, tag="nzA", bufs=4,
                                 name=f"adet{ch}")
                nc.scalar.activation(
                    adet[:], inn16[:], AF.Abs, scale=par(P_BE),
                    accum_out=accs[:, ch:ch + 1])
                th = pool.tile([128, CR, W, C], F16, tag="nzA", bufs=4,
                               name=f"th{ch}")
                nc.scalar.activation(
                    th[:], adet[:], AF.Tanh, bias=par(P_KTB), scale=par(P_KT))
                d1 = pool.tile([128, CR, W, C], F32, tag="d1", bufs=1,
                               name=f"d1_{ch}")
                if d1_eng == "act":
                    nc.scalar.activation(
                        d1[:], x_int, AF.Identity, bias=par(P_OFFGT),
                        scale=par(P_IGT))
                else:
                    d1e = nc.gpsimd if d1_eng == "pool" else nc.vector
                    d1e.tensor_scalar(
                        d1[:], x_int, par(P_IGT), par(P_OFFGT), ALU.mult, ALU.add)
                r1 = pool.tile([128, CR, W, C], F32, tag="r1", bufs=1,
                               name=f"r1_{ch}")
                nc.vector.reciprocal_approx_fast(
                    r1[:].rearrange("p a b c -> p (a b c)"),
                    d1[:].rearrange("p a b c -> p (a b c)"))
                ne0 = pool.tile([128, CR, W, C], F16, tag="nzB", bufs=3,
                                name=f"ne0_{ch}")
                neng = nc.gpsimd if ne0_eng == "pool" else nc.vector
                neng.tensor_tensor(ne0[:], adet[:], r1[:], ALU.mult)
                neq = pool.tile([128, CR, W, C], F16, tag="nzB", bufs=3,
                                name=f"neq{ch}")
                nc.vector.tensor_scalar(
                    neq[:], ne0[:], par(P_CLIP), None, ALU.min, ALU.add,
                    accum_out=accs[:, CHUNKS + ch:CHUNKS + ch + 1])
                sqn = pool.tile([128, CR, W, C], F16, tag="nzB", bufs=3,
                                name=f"sqn{ch}")
                nc.scalar.activation(sqn[:], neq[:], AF.Square)
                ee = pool.tile([128, CR, W, C], F16, tag="nzB", bufs=3,
                               name=f"ee{ch}")
                nc.scalar.activation(ee[:], sqn[:], AF.Exp, scale=-1.0)
                t2 = pool.tile([128, CR, W, C], F16, tag="nzB", bufs=3,
                               name=f"t2_{ch}")
                nc.vector.tensor_scalar(
                    t2[:], ee[:], par(P_NSQL), par(P_SQL), ALU.mult, ALU.add)
                nm = pool.tile([128, CR, W, C], F16, tag="nzA", bufs=4,
                               name=f"nm{ch}")
                nc.scalar.activation(nm[:], t2[:], AF.Square)
                s3 = pool.tile([128, CR, W, C], F16, tag="nzA", bufs=4,
                               name=f"s3_{ch}")
                if s3_split:
                    thp1 = pool.tile([128, CR, W, C], F16, tag="nzB", bufs=3,
                                     name=f"thp1{ch}")
                    nc.vector.tensor_scalar(thp1[:], th[:], 1.0, None, ALU.add)
                    nc.vector.tensor_tensor(s3[:], thp1[:], nm[:], ALU.mult)
                else:
                    nc.vector.scalar_tensor_tensor(
                        s3[:], th[:], 1.0, nm[:], ALU.add, ALU.mult)
                sharp = pool.tile([128, CR, W, C], F16, tag="sharp", bufs=2,
                                  name=f"sharp{ch}")
                nc.vector.tensor_tensor(sharp[:], s3[:], inn16[:], ALU.mult)
                tts = pool.tile([128, CR, W, C], F16, tag="tts", bufs=CHUNKS,
                                name=f"tts{ch}")
                (nc.gpsimd if tts_eng == "pool" else nc.vector).tensor_tensor(
                    tts[:], tT[:], sharp[:], ALU.add)
                tts_s.append(tts)

              # ---------------- skip flags ----------------
              my2 = pool.tile([128, 2], F32, tag="fl", bufs=1, name=f"my2{rep}")
              t_a = pool.tile([128, 2], F32, tag="fl2", bufs=1, name=f"ta{rep}")
              nc.vector.tensor_tensor(t_a[:], accs[:, 0:2], accs[:, 2:4], ALU.add)
              nc.vector.tensor_tensor(my2[:, 0:1], t_a[:, 0:1], t_a[:, 1:2], ALU.add)
              t_n = pool.tile([128, 2], F32, tag="fl3", bufs=1, name=f"tn{rep}")
              nc.vector.tensor_tensor(
                  t_n[:], accs[:, CHUNKS:CHUNKS + 2], accs[:, CHUNKS + 2:CHUNKS + 4],
                  ALU.add)
              nc.vector.tensor_tensor(my2[:, 1:2], t_n[:, 0:1], t_n[:, 1:2], ALU.add)
              other2 = pool.tile([128, 2], F32, tag="fl4", bufs=1, name=f"oth{rep}")
              nc.vector.stream_shuffle(other2[:], my2[:], [i ^ 1 for i in range(32)])
              tot = pool.tile([128, 2], F32, tag="fl5", bufs=1, name=f"tot{rep}")
              nc.vector.tensor_tensor(tot[:], my2[:], other2[:], ALU.add)
              fa = pool.tile([128, 1], F32, tag="fl6", bufs=1, name=f"fa{rep}")
              nc.vector.tensor_scalar(
                  fa[:], tot[:, 0:1], MEAN_N * SKIP_THRESH, None, ALU.is_lt)
              fn = pool.tile([128, 1], F32, tag="fl7", bufs=1, name=f"fn{rep}")
              nc.vector.tensor_scalar(fn[:], tot[:, 1:2], par(P_TN), None, ALU.is_lt)
              fl = pool.tile([128, 1], F32, tag="fl8", bufs=1, name=f"fl{rep}")
              nc.vector.tensor_tensor(fl[:], fa[:], fn[:], ALU.max)
              w1 = pool.tile([128, 1], F32, tag="fl9", bufs=1, name=f"w1{rep}")
              nc.vector.tensor_scalar(w1[:], fl[:], -1.0, 1.0, ALU.mult, ALU.add)

              # ---------------- blend + clip + store ----------------
              for ch in range(CHUNKS):
                xre = pool.tile([128, CR, W, C], F32, tag="xre",
                                bufs=int(cfg.get("xre_bufs", 2)),
                                name=f"xre{ch}_{rep}")
                nc.sync.dma_start(
                    xre[:], xdram[:, ch * CR + 1:ch * CR + 1 + CR, 1:65, :])
                o2b = pool.tile([128, CR, W, C], F32, tag="o2b", bufs=1,
                                name=f"o2b{ch}_{rep}")
                o2e = nc.gpsimd if o2b_eng == "pool" else nc.vector
                o2e.scalar_tensor_tensor(
                    o2b[:], tts_s[ch][:], w1[:], xre[:], ALU.mult, ALU.add)
                o3 = pool.tile([128, CR, W, C], F32, tag="o3",
                               bufs=int(cfg.get("o3_bufs", 2)),
                               name=f"o3_{ch}_{rep}")
                (nc.gpsimd if o3_eng == "pool" else nc.vector).tensor_scalar(
                    o3[:], o2b[:], 1e-5, 1.0, ALU.max, ALU.min)
                nc.sync.dma_start(odram[:, ch * CR:(ch + 1) * CR, :, :], o3[:])

    nc.compile()
    return nc


def _get_program(cfg=None):
    key = tuple(sorted((cfg or {}).items()))
    if key not in _CACHE:
        _CACHE[key] = build_program(cfg)
    return _CACHE[key]


# --------------------------------------------------------------------------
# entry point
# --------------------------------------------------------------------------

def kernel(images, params, k):
    from concourse.bass_utils import run_bass_kernel_spmd

    nc = _get_program({})
    in_maps = _host_prep(np.asarray(images), np.asarray(params), np.asarray(k))
    res = run_bass_kernel_spmd(nc, in_maps, list(range(N_CORES)))
    return _host_post(res.results, images, params).astype(np.float32)

